# revision 13
# baseline (speedup 1.0000x reference)
import sys

sys.path.insert(0, "/opt/trn_rl_repo")

import numpy as np

P = 128          # partitions / tile edge
D = 128          # model dim
H = 4            # heads
DH = 32          # head dim
NCORES = 8

# Full-problem geometry (N=100000, E=800000). Each core owns NBLK node
# blocks of 128 nodes; every block's incident-edge list is padded to
# TBLK tiles of 128 edges so the SPMD program is uniform across cores.
NBLK_FULL = 98                      # 98*128 = 12544 own nodes/core
NPAD_FULL = NCORES * NBLK_FULL * P  # 100352 padded nodes


def _channel_perm():
    # torch reshape (N, DH, H): flat channel c = d*H + h. We relayout to
    # h-major c' = h*DH + d by permuting weight rows: perm[c'] = d*H + h.
    cp = np.arange(D)
    return (cp % DH) * H + (cp // DH)


def _build_program(NPAD, NOWN, NBLK, TBLK):
    import concourse.bass as bass
    import concourse.tile as tile
    from concourse import bacc, mybir
    from concourse.masks import make_identity
    from contextlib import ExitStack

    dt = mybir.dt
    f32, f16, bf16, i32 = dt.float32, dt.float16, dt.bfloat16, dt.int32
    NT = NBLK * TBLK      # edge tiles per core
    XT = NPAD // P        # x tiles for k/v projection (all nodes)
    QT = NOWN // P        # x tiles for q projection (own nodes) == NBLK

    nc = bacc.Bacc("TRN2", target_bir_lowering=False, debug=False,
                   num_devices=NCORES)

    # x ships host-transposed (channel-major) so the contraction dim is
    # already on partitions: no PE transpose needed anywhere.
    xt_d = nc.dram_tensor("xt", [D, NPAD], f32, kind="ExternalInput").ap()
    xot_d = nc.dram_tensor("xot", [D, NOWN], f32, kind="ExternalInput").ap()
    wkv_d = nc.dram_tensor("wkv", [D, 2 * D], f16, kind="ExternalInput").ap()
    wq_d = nc.dram_tensor("wq", [D, D], f16, kind="ExternalInput").ap()
    wo_d = nc.dram_tensor("wo", [D, D], f16, kind="ExternalInput").ap()
    bq_d = nc.dram_tensor("bq", [1, D], f16, kind="ExternalInput").ap()
    bo_d = nc.dram_tensor("bo", [1, D], f16, kind="ExternalInput").ap()
    ci_d = nc.dram_tensor("ci", [P, NT], i32, kind="ExternalInput").ap()
    IW = TBLK * P // 16  # int16 index words per block (16-partition wrap)
    qi_d = nc.dram_tensor("qi", [P, NBLK * IW], dt.int16,
                          kind="ExternalInput").ap()
    rl_d = nc.dram_tensor("rl", [P, NT], f16, kind="ExternalInput").ap()
    io_d = nc.dram_tensor("io", [P, P], f16, kind="ExternalInput").ap()

    out_d = nc.dram_tensor("out", [NOWN, D], f32, kind="ExternalOutput").ap()
    kv_d = nc.dram_tensor("kv", [NPAD, 2 * D], f16).ap()
    q_d = nc.dram_tensor("q", [NOWN, D], f16).ap()

    AF = mybir.ActivationFunctionType
    OP = mybir.AluOpType

    with tile.TileContext(nc) as tc, ExitStack() as ctx:
        res = ctx.enter_context(tc.tile_pool(name="res", bufs=1))
        wkv_sb = res.tile([D, 2 * D], f16, name="wkv_sb")
        wq_sb = res.tile([D, D], f16, name="wq_sb")
        wo_sb = res.tile([D, D], f16, name="wo_sb")
        bq_sb = res.tile([1, D], f16, name="bq_sb")
        bo_sb = res.tile([1, D], f16, name="bo_sb")
        ci_sb = res.tile([P, NT], i32, name="ci_sb")
        qi_sb = res.tile([P, NBLK * IW], dt.int16, name="qi_sb")
        rl_sb = res.tile([P, NT], f16, name="rl_sb")
        io_sb = res.tile([P, P], f16, name="io_sb")
        ones_sb = res.tile([1, P], f16, name="ones_sb")
        ident = res.tile([P, P], f16, name="ident")

        for sb_t, dr_t in [(wkv_sb, wkv_d), (wq_sb, wq_d), (wo_sb, wo_d),
                           (bq_sb, bq_d), (bo_sb, bo_d), (ci_sb, ci_d),
                           (qi_sb, qi_d), (rl_sb, rl_d), (io_sb, io_d)]:
            nc.sync.dma_start(sb_t[:], dr_t[:])
        nc.vector.memset(ones_sb[:], 1.0)
        make_identity(nc, ident[:])

        # ---- phase A: k/v for all nodes, q for own nodes ----
        # bk is dropped entirely (softmax shift-invariance) and bv is
        # folded into bo on the host (attention weights sum to 1), so
        # the kv projection is a single matmul per tile.
        with tc.tile_pool(name="xa", bufs=3) as xa, \
             tc.tile_pool(name="pa", bufs=2, space="PSUM") as pa:
            for i in range(XT):
                x32 = xa.tile([P, P], f32, name="x32")
                nc.scalar.dma_start(x32[:], xt_d[:, i * P:(i + 1) * P])
                x16 = xa.tile([P, P], f16, name="x16")
                nc.vector.tensor_copy(x16[:], x32[:])
                kv_ps = pa.tile([P, 2 * D], f32, name="kv_ps")
                nc.tensor.matmul(kv_ps[:], lhsT=x16[:], rhs=wkv_sb[:],
                                 start=True, stop=True)
                kv_sb = xa.tile([P, 2 * D], f16, name="kv_sb")
                nc.scalar.copy(kv_sb[:], kv_ps[:])
                nc.scalar.dma_start(kv_d[i * P:(i + 1) * P, :], kv_sb[:])

            for j in range(QT):
                xo32 = xa.tile([P, P], f32, name="xo32")
                nc.scalar.dma_start(xo32[:], xot_d[:, j * P:(j + 1) * P])
                xo16 = xa.tile([P, P], f16, name="xo16")
                nc.vector.tensor_copy(xo16[:], xo32[:])
                q_ps = pa.tile([P, D], f32, name="q_ps")
                nc.tensor.matmul(q_ps[:], lhsT=ones_sb[:], rhs=bq_sb[:],
                                 start=True, stop=False)
                nc.tensor.matmul(q_ps[:], lhsT=xo16[:], rhs=wq_sb[:],
                                 start=False, stop=True)
                q_sb = xa.tile([P, D], f16, name="q_sb")
                nc.scalar.copy(q_sb[:], q_ps[:])
                nc.scalar.dma_start(q_d[j * P:(j + 1) * P, :], q_sb[:])

        # ---- phase B: per-block batched gather + scores + aggregation ----
        with tc.tile_pool(name="eg", bufs=3) as eg, \
             tc.tile_pool(name="ep", bufs=2, space="PSUM") as ep, \
             tc.tile_pool(name="yp", bufs=2, space="PSUM") as yp:
            for b in range(NBLK):
                T0 = b * TBLK
                kv_g = eg.tile([P, TBLK, 2 * D], f16, name="kv_g")
                for t in range(TBLK):
                    nc.gpsimd.indirect_dma_start(
                        out=kv_g[:, t, :], out_offset=None, in_=kv_d[:],
                        in_offset=bass.IndirectOffsetOnAxis(
                            ap=ci_sb[:, T0 + t:T0 + t + 1], axis=0))
                q_g = eg.tile([P, TBLK, D], f16, name="q_g")
                # dma_gather tops out at 1024 descriptors per instruction
                for t0 in range(0, TBLK, 8):
                    t1 = min(t0 + 8, TBLK)
                    ni = (t1 - t0) * P
                    nc.gpsimd.dma_gather(
                        out_ap=q_g[:, t0:t1, :], in_ap=q_d[:],
                        idxs_ap=qi_sb[:, b * IW + t0 * (P // 16):
                                      b * IW + t0 * (P // 16) + ni // 16],
                        num_idxs=ni, num_idxs_reg=ni, elem_size=D)

                sel = eg.tile([P, TBLK, P], bf16, name="sel")
                nc.vector.tensor_tensor(
                    out=sel[:],
                    in0=rl_sb[:, T0:T0 + TBLK].to_broadcast((P, TBLK, P)),
                    in1=io_sb[:][:, None, :].to_broadcast((P, TBLK, P)),
                    op=OP.is_equal)
                prod = eg.tile([P, TBLK, D], f16, name="prod")
                nc.vector.tensor_tensor(out=prod[:], in0=q_g[:],
                                        in1=kv_g[:, :, 0:D], op=OP.mult)
                s_b = eg.tile([P, TBLK, H], f32, name="s_b")
                nc.vector.tensor_reduce(
                    out=s_b[:],
                    in_=prod[:].rearrange("p t (h d) -> p t h d", h=H),
                    axis=mybir.AxisListType.X, op=OP.add)
                wext = eg.tile([P, TBLK, D + H], bf16, name="wext")
                nc.scalar.activation(wext[:, :, D:D + H], s_b[:], AF.Exp)
                nc.vector.tensor_tensor(
                    out=wext[:, :, 0:D].rearrange("p t (h d) -> p t h d", h=H),
                    in0=kv_g[:, :, D:2 * D].rearrange(
                        "p t (h d) -> p t h d", h=H),
                    in1=wext[:, :, D:D + H].to_broadcast((P, TBLK, H, DH)),
                    op=OP.mult)

                ypre = yp.tile([P, D + H], f32, name="ypre")
                for t in range(TBLK):
                    nc.tensor.matmul(ypre[:], lhsT=sel[:, t, :],
                                     rhs=wext[:, t, :],
                                     start=(t == 0), stop=(t == TBLK - 1))

                zr = eg.tile([P, H], f32, name="zr")
                nc.vector.tensor_scalar_add(zr[:], ypre[:, D:D + H], 1e-30)
                rz = eg.tile([P, H], f32, name="rz")
                nc.vector.reciprocal(rz[:], zr[:])
                yb = eg.tile([P, D], f16, name="yb")
                nc.vector.tensor_tensor(
                    out=yb[:].rearrange("p (h d) -> p h d", h=H),
                    in0=ypre[:, 0:D].rearrange("p (h d) -> p h d", h=H),
                    in1=rz[:].to_broadcast((P, H, DH)),
                    op=OP.mult)
                yT_ps = ep.tile([P, D], f16, name="yT_ps")
                nc.tensor.transpose(yT_ps[:], yb[:], ident[:])
                yT = eg.tile([P, D], f16, name="yT")
                nc.scalar.copy(yT[:], yT_ps[:])
                o_ps = ep.tile([P, D], f32, name="o_ps")
                nc.tensor.matmul(o_ps[:], lhsT=ones_sb[:], rhs=bo_sb[:],
                                 start=True, stop=False)
                nc.tensor.matmul(o_ps[:], lhsT=yT[:], rhs=wo_sb[:],
                                 start=False, stop=True)
                o_sb = eg.tile([P, D], f32, name="o_sb")
                nc.scalar.copy(o_sb[:], o_ps[:])
                nc.scalar.dma_start(out_d[b * P:(b + 1) * P, :], o_sb[:])

    nc.compile()
    return nc


def _prepare_inputs(x, row, col, Wq, bq, Wk, bk, Wv, bv, Wo, bo,
                    NPAD, NOWN, NBLK, TBLK):
    """Host-side sharding: per-core padded edge lists + permuted weights."""
    N = x.shape[0]
    perm = _channel_perm()
    s = np.sqrt(float(H))
    wkv_in = np.ascontiguousarray(
        np.concatenate([Wk[perm, :].T, Wv[perm, :].T], axis=1)
    ).astype(np.float16)
    wq_in = np.ascontiguousarray((Wq[perm, :] / s).T).astype(np.float16)
    wo_in = np.ascontiguousarray(Wo[:, perm].T).astype(np.float16)
    bq_in = (bq[perm] / s).reshape(1, D).astype(np.float16)
    # bv folds through the output projection exactly: sum_e a_e = 1.
    bo_in = (bo + Wo @ bv).reshape(1, D).astype(np.float16)
    io_in = np.tile(np.arange(P, dtype=np.float16), (P, 1))

    x_pad = np.zeros((NPAD, D), np.float32)
    x_pad[:N] = x
    xt_in = np.ascontiguousarray(x_pad.T)

    NT = NBLK * TBLK
    EPC = NT * P  # padded edges per core
    in_maps = []
    for c in range(NCORES):
        lo, hi = c * NOWN, (c + 1) * NOWN
        e0 = np.searchsorted(row, lo, "left")
        e1 = np.searchsorted(row, hi, "left")
        rows_c = (row[e0:e1] - lo).astype(np.int64)
        cols_c = col[e0:e1].astype(np.int64)
        blk = rows_c // P
        blk_starts = np.searchsorted(blk, np.arange(NBLK), "left")
        rank = np.arange(rows_c.shape[0]) - blk_starts[blk]
        cnts = np.bincount(blk, minlength=NBLK)
        if cnts.max() > TBLK * P:
            raise ValueError(f"TBLK={TBLK} too small: need "
                             f"{int(np.ceil(cnts.max() / P))}")
        pos = blk * (TBLK * P) + rank
        ci = np.zeros(EPC, np.int32)
        qi = np.zeros(EPC, np.int16)
        rl = np.full(EPC, -1.0, np.float16)
        ci[pos] = cols_c.astype(np.int32)
        qi[pos] = rows_c.astype(np.int16)
        rl[pos] = (rows_c % P).astype(np.float16)
        # dma_gather index layout: linear slot i at [i % 16, i // 16],
        # replicated across the 8 Q7-core partition groups.
        qi_w = qi.reshape(NBLK, TBLK * P // 16, 16).transpose(0, 2, 1)
        qi_w = np.tile(qi_w, (1, 8, 1)).transpose(1, 0, 2).reshape(
            P, NBLK * (TBLK * P // 16))
        in_maps.append({
            "xt": xt_in,
            "xot": np.ascontiguousarray(x_pad[lo:hi].T),
            "wkv": wkv_in, "wq": wq_in, "wo": wo_in,
            "bq": bq_in, "bo": bo_in,
            "ci": np.ascontiguousarray(ci.reshape(NT, P).T),
            "qi": np.ascontiguousarray(qi_w),
            "rl": np.ascontiguousarray(rl.reshape(NT, P).T),
            "io": io_in,
        })
    return in_maps


def _required_tblk(row, NOWN, NBLK):
    row = np.asarray(row, np.int64)
    need = 1
    for c in range(NCORES):
        lo, hi = c * NOWN, (c + 1) * NOWN
        e0 = np.searchsorted(row, lo, "left")
        e1 = np.searchsorted(row, hi, "left")
        blk = (row[e0:e1] - lo) // P
        cnts = np.bincount(blk, minlength=NBLK)
        need = max(need, int(np.ceil(cnts.max() / P)))
    return need


def _install_ntff_hook():
    """The agent image's antenv lacks axon_hooks; inject it so trace=True
    can drive NTFF profiling through libaxon_pjrt.so."""
    import importlib
    try:
        importlib.import_module("antenv.axon_hooks")
        return
    except ImportError:
        pass
    import types
    if "/root/.axon_site" not in sys.path:
        sys.path.insert(0, "/root/.axon_site")
    from trn_agent_boot.trn_boot import _ntff_profile_via_ctypes
    hook = _ntff_profile_via_ctypes("/opt/axon/libaxon_pjrt.so")
    mod = types.ModuleType("antenv.axon_hooks")
    state = {"hook": hook}
    mod.get_axon_ntff_profile_hook = lambda: state["hook"]
    mod.set_axon_ntff_profile_hook = lambda h: state.update(hook=h)
    import antenv
    antenv.axon_hooks = mod
    sys.modules["antenv.axon_hooks"] = mod


def run(x, row, col, Wq, bq, Wk, bk, Wv, bv, Wo, bo, NBLK=NBLK_FULL,
        trace=False, tmpdir=None):
    from concourse import bass_utils
    from concourse.bass_utils import run_bass_kernel_spmd
    if trace:
        _install_ntff_hook()
        bass_utils.upload_artifacts = lambda d: "local://" + d

    x = np.asarray(x, np.float32)
    row = np.asarray(row, np.int64)
    col = np.asarray(col, np.int64)
    N = x.shape[0]
    NOWN = NBLK * P
    NPAD = NCORES * NOWN
    assert NPAD >= N
    TBLK = _required_tblk(row, NOWN, NBLK)
    nc = _build_program(NPAD, NOWN, NBLK, TBLK)
    in_maps = _prepare_inputs(
        x, row, col,
        np.asarray(Wq, np.float32), np.asarray(bq, np.float32),
        np.asarray(Wk, np.float32), np.asarray(bk, np.float32),
        np.asarray(Wv, np.float32), np.asarray(bv, np.float32),
        np.asarray(Wo, np.float32), np.asarray(bo, np.float32),
        NPAD, NOWN, NBLK, TBLK)
    res = run_bass_kernel_spmd(nc, in_maps, list(range(NCORES)), trace=trace,
                               tmpdir=tmpdir)
    out = np.concatenate([res.results[c]["out"] for c in range(NCORES)], 0)
    return out[:N].astype(np.float32), res


def kernel(**inputs):
    out, _ = run(**inputs)
    return out


# revision 15
# speedup vs baseline: 1.3608x; 1.3608x over previous
import sys

sys.path.insert(0, "/opt/trn_rl_repo")

import numpy as np

P = 128          # partitions / tile edge
D = 128          # model dim
H = 4            # heads
DH = 32          # head dim
NCORES = 8

# Full-problem geometry (N=100000, E=800000). Each core owns NBLK node
# blocks of 128 nodes; every block's incident-edge list is padded to
# TBLK tiles of 128 edges so the SPMD program is uniform across cores.
NBLK_FULL = 98                      # 98*128 = 12544 own nodes/core
NPAD_FULL = NCORES * NBLK_FULL * P  # 100352 padded nodes


def _channel_perm():
    # torch reshape (N, DH, H): flat channel c = d*H + h. We relayout to
    # h-major c' = h*DH + d by permuting weight rows: perm[c'] = d*H + h.
    cp = np.arange(D)
    return (cp % DH) * H + (cp // DH)


def _build_program(NPAD, NOWN, NBLK, TBLK):
    import concourse.bass as bass
    import concourse.tile as tile
    from concourse import bacc, mybir
    from concourse.masks import make_identity
    from contextlib import ExitStack

    dt = mybir.dt
    f32, f16, bf16, i32 = dt.float32, dt.float16, dt.bfloat16, dt.int32
    NT = NBLK * TBLK      # edge tiles per core
    XT = NPAD // P        # x tiles for k/v projection (all nodes)
    QT = NOWN // P        # x tiles for q projection (own nodes) == NBLK

    nc = bacc.Bacc("TRN2", target_bir_lowering=False, debug=False,
                   num_devices=NCORES)

    # x ships host-transposed (channel-major) so the contraction dim is
    # already on partitions: no PE transpose needed anywhere.
    xt_d = nc.dram_tensor("xt", [D, NPAD], f32, kind="ExternalInput").ap()
    xot_d = nc.dram_tensor("xot", [D, NOWN], f32, kind="ExternalInput").ap()
    wkv_d = nc.dram_tensor("wkv", [D, 2 * D], f16, kind="ExternalInput").ap()
    wq_d = nc.dram_tensor("wq", [D, D], f16, kind="ExternalInput").ap()
    wo_d = nc.dram_tensor("wo", [D, D], f16, kind="ExternalInput").ap()
    bq_d = nc.dram_tensor("bq", [1, D], f16, kind="ExternalInput").ap()
    bo_d = nc.dram_tensor("bo", [1, D], f16, kind="ExternalInput").ap()
    ci_d = nc.dram_tensor("ci", [P, NT], i32, kind="ExternalInput").ap()
    IW = TBLK * P // 16  # int16 index words per block (16-partition wrap)
    qi_d = nc.dram_tensor("qi", [P, NBLK * IW], dt.int16,
                          kind="ExternalInput").ap()
    rl_d = nc.dram_tensor("rl", [P, NT], f16, kind="ExternalInput").ap()
    io_d = nc.dram_tensor("io", [P, P], f16, kind="ExternalInput").ap()

    out_d = nc.dram_tensor("out", [NOWN, D], f32, kind="ExternalOutput").ap()
    kv_d = nc.dram_tensor("kv", [NPAD, 2 * D], f16).ap()
    q_d = nc.dram_tensor("q", [NOWN, D], f16).ap()
    qg_d = nc.dram_tensor("qg", [NBLK, P, TBLK * D], f16).ap()

    AF = mybir.ActivationFunctionType
    OP = mybir.AluOpType

    with tile.TileContext(nc) as tc, ExitStack() as ctx:
        res = ctx.enter_context(tc.tile_pool(name="res", bufs=1))
        wkv_sb = res.tile([D, 2 * D], f16, name="wkv_sb")
        wq_sb = res.tile([D, D], f16, name="wq_sb")
        wo_sb = res.tile([D, D], f16, name="wo_sb")
        bq_sb = res.tile([1, D], f16, name="bq_sb")
        bo_sb = res.tile([1, D], f16, name="bo_sb")
        ci_sb = res.tile([P, NT], i32, name="ci_sb")
        qi_sb = res.tile([P, NBLK * IW], dt.int16, name="qi_sb")
        rl_sb = res.tile([P, NT], f16, name="rl_sb")
        io_sb = res.tile([P, P], f16, name="io_sb")
        ones_sb = res.tile([1, P], f16, name="ones_sb")
        ident = res.tile([P, P], f16, name="ident")

        for sb_t, dr_t in [(wkv_sb, wkv_d), (wq_sb, wq_d), (wo_sb, wo_d),
                           (bq_sb, bq_d), (bo_sb, bo_d), (ci_sb, ci_d),
                           (qi_sb, qi_d), (rl_sb, rl_d), (io_sb, io_d)]:
            nc.sync.dma_start(sb_t[:], dr_t[:])
        nc.vector.memset(ones_sb[:], 1.0)
        make_identity(nc, ident[:])

        CH = 4  # x tiles per DMA chunk
        with tc.tile_pool(name="xa", bufs=3) as xa, \
             tc.tile_pool(name="qg", bufs=3) as qg, \
             tc.tile_pool(name="pa", bufs=2, space="PSUM") as pa:
            # ---- q projection first: q_d is ready early so the q-gather
            # staging below overlaps the (long) kv projection loop.
            for j0 in range(0, QT, CH):
                c = min(CH, QT - j0)
                xo32 = xa.tile([P, c * P], f32, name="xo32")
                nc.sync.dma_start(xo32[:], xot_d[:, j0 * P:(j0 + c) * P])
                xo16 = xa.tile([P, c * P], f16, name="xo16")
                nc.vector.tensor_copy(xo16[:], xo32[:])
                q4 = xa.tile([P, c, D], f16, name="q4")
                for t in range(c):
                    q_ps = pa.tile([P, D], f32, name="q_ps")
                    nc.tensor.matmul(q_ps[:], lhsT=ones_sb[:], rhs=bq_sb[:],
                                     start=True, stop=False)
                    nc.tensor.matmul(q_ps[:],
                                     lhsT=xo16[:, t * P:(t + 1) * P],
                                     rhs=wq_sb[:], start=False, stop=True)
                    nc.scalar.copy(q4[:, t, :], q_ps[:])
                nc.scalar.dma_start(
                    q_d[j0 * P:(j0 + c) * P, :].rearrange(
                        "(t p) c -> p t c", p=P), q4[:])

            # ---- q-gather staging: runs on the Pool engine while the
            # HWDGE/PE/DVE pipeline below computes kv. dma_gather caps at
            # 1024 descriptors per instruction.
            for b in range(NBLK):
                qgt = qg.tile([P, TBLK, D], f16, name="qgt")
                for t0 in range(0, TBLK, 8):
                    t1 = min(t0 + 8, TBLK)
                    ni = (t1 - t0) * P
                    nc.gpsimd.dma_gather(
                        out_ap=qgt[:, t0:t1, :], in_ap=q_d[:],
                        idxs_ap=qi_sb[:, b * IW + t0 * (P // 16):
                                      b * IW + t0 * (P // 16) + ni // 16],
                        num_idxs=ni, num_idxs_reg=ni, elem_size=D)
                nc.scalar.dma_start(
                    qg_d[b, :, :].rearrange("p (t c) -> p t c", t=TBLK),
                    qgt[:])

            # ---- kv projection for all nodes ----
            for i0 in range(0, XT, CH):
                c = min(CH, XT - i0)
                x32 = xa.tile([P, c * P], f32, name="x32")
                nc.sync.dma_start(x32[:], xt_d[:, i0 * P:(i0 + c) * P])
                x16 = xa.tile([P, c * P], f16, name="x16")
                nc.vector.tensor_copy(x16[:], x32[:])
                kv4 = xa.tile([P, c, 2 * D], f16, name="kv4")
                for t in range(c):
                    kv_ps = pa.tile([P, 2 * D], f32, name="kv_ps")
                    nc.tensor.matmul(kv_ps[:],
                                     lhsT=x16[:, t * P:(t + 1) * P],
                                     rhs=wkv_sb[:], start=True, stop=True)
                    nc.scalar.copy(kv4[:, t, :], kv_ps[:])
                nc.scalar.dma_start(
                    kv_d[i0 * P:(i0 + c) * P, :].rearrange(
                        "(t p) c -> p t c", p=P), kv4[:])

        # ---- phase B: per-block kv gather + scores + aggregation ----
        with tc.tile_pool(name="eg", bufs=3) as eg, \
             tc.tile_pool(name="ep", bufs=2, space="PSUM") as ep, \
             tc.tile_pool(name="yp", bufs=2, space="PSUM") as yp:
            for b in range(NBLK):
                T0 = b * TBLK
                kv_g = eg.tile([P, TBLK, 2 * D], f16, name="kv_g")
                for t in range(TBLK):
                    nc.gpsimd.indirect_dma_start(
                        out=kv_g[:, t, :], out_offset=None, in_=kv_d[:],
                        in_offset=bass.IndirectOffsetOnAxis(
                            ap=ci_sb[:, T0 + t:T0 + t + 1], axis=0))
                q_g = eg.tile([P, TBLK, D], f16, name="q_g")
                nc.sync.dma_start(
                    q_g[:], qg_d[b, :, :].rearrange("p (t c) -> p t c",
                                                    t=TBLK))

                sel = eg.tile([P, TBLK, P], bf16, name="sel")
                nc.vector.tensor_tensor(
                    out=sel[:],
                    in0=rl_sb[:, T0:T0 + TBLK].to_broadcast((P, TBLK, P)),
                    in1=io_sb[:][:, None, :].to_broadcast((P, TBLK, P)),
                    op=OP.is_equal)
                prod = eg.tile([P, TBLK, D], f16, name="prod")
                nc.vector.tensor_tensor(out=prod[:], in0=q_g[:],
                                        in1=kv_g[:, :, 0:D], op=OP.mult)
                s_b = eg.tile([P, TBLK, H], f32, name="s_b")
                nc.vector.tensor_reduce(
                    out=s_b[:],
                    in_=prod[:].rearrange("p t (h d) -> p t h d", h=H),
                    axis=mybir.AxisListType.X, op=OP.add)
                wext = eg.tile([P, TBLK, D + H], bf16, name="wext")
                nc.scalar.activation(wext[:, :, D:D + H], s_b[:], AF.Exp)
                nc.vector.tensor_tensor(
                    out=wext[:, :, 0:D].rearrange("p t (h d) -> p t h d", h=H),
                    in0=kv_g[:, :, D:2 * D].rearrange(
                        "p t (h d) -> p t h d", h=H),
                    in1=wext[:, :, D:D + H].to_broadcast((P, TBLK, H, DH)),
                    op=OP.mult)

                ypre = yp.tile([P, D + H], f32, name="ypre")
                for t in range(TBLK):
                    nc.tensor.matmul(ypre[:], lhsT=sel[:, t, :],
                                     rhs=wext[:, t, :],
                                     start=(t == 0), stop=(t == TBLK - 1))

                zr = eg.tile([P, H], f32, name="zr")
                nc.vector.tensor_scalar_add(zr[:], ypre[:, D:D + H], 1e-30)
                rz = eg.tile([P, H], f32, name="rz")
                nc.vector.reciprocal(rz[:], zr[:])
                yb = eg.tile([P, D], f16, name="yb")
                nc.vector.tensor_tensor(
                    out=yb[:].rearrange("p (h d) -> p h d", h=H),
                    in0=ypre[:, 0:D].rearrange("p (h d) -> p h d", h=H),
                    in1=rz[:].to_broadcast((P, H, DH)),
                    op=OP.mult)
                yT_ps = ep.tile([P, D], f16, name="yT_ps")
                nc.tensor.transpose(yT_ps[:], yb[:], ident[:])
                yT = eg.tile([P, D], f16, name="yT")
                nc.scalar.copy(yT[:], yT_ps[:])
                o_ps = ep.tile([P, D], f32, name="o_ps")
                nc.tensor.matmul(o_ps[:], lhsT=ones_sb[:], rhs=bo_sb[:],
                                 start=True, stop=False)
                nc.tensor.matmul(o_ps[:], lhsT=yT[:], rhs=wo_sb[:],
                                 start=False, stop=True)
                o_sb = eg.tile([P, D], f32, name="o_sb")
                nc.scalar.copy(o_sb[:], o_ps[:])
                nc.scalar.dma_start(out_d[b * P:(b + 1) * P, :], o_sb[:])

    nc.compile()
    return nc


def _prepare_inputs(x, row, col, Wq, bq, Wk, bk, Wv, bv, Wo, bo,
                    NPAD, NOWN, NBLK, TBLK):
    """Host-side sharding: per-core padded edge lists + permuted weights."""
    N = x.shape[0]
    perm = _channel_perm()
    s = np.sqrt(float(H))
    wkv_in = np.ascontiguousarray(
        np.concatenate([Wk[perm, :].T, Wv[perm, :].T], axis=1)
    ).astype(np.float16)
    wq_in = np.ascontiguousarray((Wq[perm, :] / s).T).astype(np.float16)
    wo_in = np.ascontiguousarray(Wo[:, perm].T).astype(np.float16)
    bq_in = (bq[perm] / s).reshape(1, D).astype(np.float16)
    # bv folds through the output projection exactly: sum_e a_e = 1.
    bo_in = (bo + Wo @ bv).reshape(1, D).astype(np.float16)
    io_in = np.tile(np.arange(P, dtype=np.float16), (P, 1))

    x_pad = np.zeros((NPAD, D), np.float32)
    x_pad[:N] = x
    xt_in = np.ascontiguousarray(x_pad.T)

    NT = NBLK * TBLK
    EPC = NT * P  # padded edges per core
    in_maps = []
    for c in range(NCORES):
        lo, hi = c * NOWN, (c + 1) * NOWN
        e0 = np.searchsorted(row, lo, "left")
        e1 = np.searchsorted(row, hi, "left")
        rows_c = (row[e0:e1] - lo).astype(np.int64)
        cols_c = col[e0:e1].astype(np.int64)
        blk = rows_c // P
        blk_starts = np.searchsorted(blk, np.arange(NBLK), "left")
        rank = np.arange(rows_c.shape[0]) - blk_starts[blk]
        cnts = np.bincount(blk, minlength=NBLK)
        if cnts.max() > TBLK * P:
            raise ValueError(f"TBLK={TBLK} too small: need "
                             f"{int(np.ceil(cnts.max() / P))}")
        pos = blk * (TBLK * P) + rank
        ci = np.zeros(EPC, np.int32)
        qi = np.zeros(EPC, np.int16)
        rl = np.full(EPC, -1.0, np.float16)
        ci[pos] = cols_c.astype(np.int32)
        qi[pos] = rows_c.astype(np.int16)
        rl[pos] = (rows_c % P).astype(np.float16)
        # dma_gather index layout: linear slot i at [i % 16, i // 16],
        # replicated across the 8 Q7-core partition groups.
        qi_w = qi.reshape(NBLK, TBLK * P // 16, 16).transpose(0, 2, 1)
        qi_w = np.tile(qi_w, (1, 8, 1)).transpose(1, 0, 2).reshape(
            P, NBLK * (TBLK * P // 16))
        in_maps.append({
            "xt": xt_in,
            "xot": np.ascontiguousarray(x_pad[lo:hi].T),
            "wkv": wkv_in, "wq": wq_in, "wo": wo_in,
            "bq": bq_in, "bo": bo_in,
            "ci": np.ascontiguousarray(ci.reshape(NT, P).T),
            "qi": np.ascontiguousarray(qi_w),
            "rl": np.ascontiguousarray(rl.reshape(NT, P).T),
            "io": io_in,
        })
    return in_maps


def _required_tblk(row, NOWN, NBLK):
    row = np.asarray(row, np.int64)
    need = 1
    for c in range(NCORES):
        lo, hi = c * NOWN, (c + 1) * NOWN
        e0 = np.searchsorted(row, lo, "left")
        e1 = np.searchsorted(row, hi, "left")
        blk = (row[e0:e1] - lo) // P
        cnts = np.bincount(blk, minlength=NBLK)
        need = max(need, int(np.ceil(cnts.max() / P)))
    return need


def _install_ntff_hook():
    """The agent image's antenv lacks axon_hooks; inject it so trace=True
    can drive NTFF profiling through libaxon_pjrt.so."""
    import importlib
    try:
        importlib.import_module("antenv.axon_hooks")
        return
    except ImportError:
        pass
    import types
    if "/root/.axon_site" not in sys.path:
        sys.path.insert(0, "/root/.axon_site")
    from trn_agent_boot.trn_boot import _ntff_profile_via_ctypes
    hook = _ntff_profile_via_ctypes("/opt/axon/libaxon_pjrt.so")
    mod = types.ModuleType("antenv.axon_hooks")
    state = {"hook": hook}
    mod.get_axon_ntff_profile_hook = lambda: state["hook"]
    mod.set_axon_ntff_profile_hook = lambda h: state.update(hook=h)
    import antenv
    antenv.axon_hooks = mod
    sys.modules["antenv.axon_hooks"] = mod


def run(x, row, col, Wq, bq, Wk, bk, Wv, bv, Wo, bo, NBLK=NBLK_FULL,
        trace=False, tmpdir=None):
    from concourse import bass_utils
    from concourse.bass_utils import run_bass_kernel_spmd
    if trace:
        _install_ntff_hook()
        bass_utils.upload_artifacts = lambda d: "local://" + d

    x = np.asarray(x, np.float32)
    row = np.asarray(row, np.int64)
    col = np.asarray(col, np.int64)
    N = x.shape[0]
    NOWN = NBLK * P
    NPAD = NCORES * NOWN
    assert NPAD >= N
    TBLK = _required_tblk(row, NOWN, NBLK)
    nc = _build_program(NPAD, NOWN, NBLK, TBLK)
    in_maps = _prepare_inputs(
        x, row, col,
        np.asarray(Wq, np.float32), np.asarray(bq, np.float32),
        np.asarray(Wk, np.float32), np.asarray(bk, np.float32),
        np.asarray(Wv, np.float32), np.asarray(bv, np.float32),
        np.asarray(Wo, np.float32), np.asarray(bo, np.float32),
        NPAD, NOWN, NBLK, TBLK)
    res = run_bass_kernel_spmd(nc, in_maps, list(range(NCORES)), trace=trace,
                               tmpdir=tmpdir)
    out = np.concatenate([res.results[c]["out"] for c in range(NCORES)], 0)
    return out[:N].astype(np.float32), res


def kernel(**inputs):
    out, _ = run(**inputs)
    return out


# revision 16
# speedup vs baseline: 2.1510x; 1.5807x over previous
import sys

sys.path.insert(0, "/opt/trn_rl_repo")

import numpy as np

P = 128          # partitions / tile edge
D = 128          # model dim
H = 4            # heads
DH = 32          # head dim
NCORES = 8

# Full-problem geometry (N=100000, E=800000). Each core owns NBLK node
# blocks of 128 nodes; every block's incident-edge list is padded to
# TBLK tiles of 128 edges so the SPMD program is uniform across cores.
NBLK_FULL = 98                      # 98*128 = 12544 own nodes/core
NPAD_FULL = NCORES * NBLK_FULL * P  # 100352 padded nodes


def _channel_perm():
    # torch reshape (N, DH, H): flat channel c = d*H + h. We relayout to
    # h-major c' = h*DH + d by permuting weight rows: perm[c'] = d*H + h.
    cp = np.arange(D)
    return (cp % DH) * H + (cp // DH)


def _build_program(NPAD, NOWN, NBLK, TBLK):
    import concourse.bass as bass
    import concourse.tile as tile
    from concourse import bacc, mybir
    from concourse.masks import make_identity
    from contextlib import ExitStack

    dt = mybir.dt
    f32, f16, bf16, i32 = dt.float32, dt.float16, dt.bfloat16, dt.int32
    NT = NBLK * TBLK      # edge tiles per core
    XT = NPAD // P        # x tiles for k/v projection (all nodes)
    QT = NOWN // P        # x tiles for q projection (own nodes) == NBLK

    nc = bacc.Bacc("TRN2", target_bir_lowering=False, debug=False,
                   num_devices=NCORES)

    # x ships host-transposed (channel-major) so the contraction dim is
    # already on partitions: no PE transpose needed anywhere.
    xt_d = nc.dram_tensor("xt", [D, NPAD], f32, kind="ExternalInput").ap()
    xot_d = nc.dram_tensor("xot", [D, NOWN], f32, kind="ExternalInput").ap()
    wkv_d = nc.dram_tensor("wkv", [D, 2 * D], f16, kind="ExternalInput").ap()
    wq_d = nc.dram_tensor("wq", [D, D], f16, kind="ExternalInput").ap()
    wo_d = nc.dram_tensor("wo", [D, D], f16, kind="ExternalInput").ap()
    bq_d = nc.dram_tensor("bq", [1, D], f16, kind="ExternalInput").ap()
    bo_d = nc.dram_tensor("bo", [1, D], f16, kind="ExternalInput").ap()
    ci_d = nc.dram_tensor("ci", [P, NT], i32, kind="ExternalInput").ap()
    selt_d = nc.dram_tensor("selt", [NBLK, P, TBLK * P], f16,
                            kind="ExternalInput").ap()
    rl_d = nc.dram_tensor("rl", [P, NT], f16, kind="ExternalInput").ap()
    io_d = nc.dram_tensor("io", [P, P], f16, kind="ExternalInput").ap()

    out_d = nc.dram_tensor("out", [NOWN, D], f32, kind="ExternalOutput").ap()
    kv_d = nc.dram_tensor("kv", [NPAD, 2 * D], f16).ap()
    q_d = nc.dram_tensor("q", [NOWN, D], f16).ap()


    AF = mybir.ActivationFunctionType
    OP = mybir.AluOpType

    with tile.TileContext(nc) as tc, ExitStack() as ctx:
        res = ctx.enter_context(tc.tile_pool(name="res", bufs=1))
        wkv_sb = res.tile([D, 2 * D], f16, name="wkv_sb")
        wq_sb = res.tile([D, D], f16, name="wq_sb")
        wo_sb = res.tile([D, D], f16, name="wo_sb")
        bq_sb = res.tile([1, D], f16, name="bq_sb")
        bo_sb = res.tile([1, D], f16, name="bo_sb")
        ci_sb = res.tile([P, NT], i32, name="ci_sb")

        rl_sb = res.tile([P, NT], f16, name="rl_sb")
        io_sb = res.tile([P, P], f16, name="io_sb")
        ones_sb = res.tile([1, P], f16, name="ones_sb")
        ident = res.tile([P, P], f16, name="ident")

        for sb_t, dr_t in [(wkv_sb, wkv_d), (wq_sb, wq_d), (wo_sb, wo_d),
                           (bq_sb, bq_d), (bo_sb, bo_d), (ci_sb, ci_d),
                           (rl_sb, rl_d), (io_sb, io_d)]:
            nc.sync.dma_start(sb_t[:], dr_t[:])
        nc.vector.memset(ones_sb[:], 1.0)
        make_identity(nc, ident[:])

        CH = 4  # x tiles per DMA chunk
        with tc.tile_pool(name="xa", bufs=3) as xa, \
             tc.tile_pool(name="pa", bufs=2, space="PSUM") as pa:
            # ---- q projection first: q_d is ready early so the q-gather
            # staging below overlaps the (long) kv projection loop.
            for j0 in range(0, QT, CH):
                c = min(CH, QT - j0)
                xo32 = xa.tile([P, c * P], f32, name="xo32")
                nc.sync.dma_start(xo32[:], xot_d[:, j0 * P:(j0 + c) * P])
                xo16 = xa.tile([P, c * P], f16, name="xo16")
                nc.vector.tensor_copy(xo16[:], xo32[:])
                q4 = xa.tile([P, c, D], f16, name="q4")
                for t in range(c):
                    q_ps = pa.tile([P, D], f32, name="q_ps")
                    nc.tensor.matmul(q_ps[:], lhsT=ones_sb[:], rhs=bq_sb[:],
                                     start=True, stop=False)
                    nc.tensor.matmul(q_ps[:],
                                     lhsT=xo16[:, t * P:(t + 1) * P],
                                     rhs=wq_sb[:], start=False, stop=True)
                    nc.scalar.copy(q4[:, t, :], q_ps[:])
                nc.scalar.dma_start(
                    q_d[j0 * P:(j0 + c) * P, :].rearrange(
                        "(t p) c -> p t c", p=P), q4[:])

            # ---- kv projection for all nodes ----
            for i0 in range(0, XT, CH):
                c = min(CH, XT - i0)
                x32 = xa.tile([P, c * P], f32, name="x32")
                nc.sync.dma_start(x32[:], xt_d[:, i0 * P:(i0 + c) * P])
                x16 = xa.tile([P, c * P], f16, name="x16")
                nc.vector.tensor_copy(x16[:], x32[:])
                kv4 = xa.tile([P, c, 2 * D], f16, name="kv4")
                for t in range(c):
                    kv_ps = pa.tile([P, 2 * D], f32, name="kv_ps")
                    nc.tensor.matmul(kv_ps[:],
                                     lhsT=x16[:, t * P:(t + 1) * P],
                                     rhs=wkv_sb[:], start=True, stop=True)
                    nc.vector.tensor_copy(kv4[:, t, :], kv_ps[:])
                nc.scalar.dma_start(
                    kv_d[i0 * P:(i0 + c) * P, :].rearrange(
                        "(t p) c -> p t c", p=P), kv4[:])

        # ---- phase B: per-block kv gather + scores + aggregation ----
        with tc.tile_pool(name="eg", bufs=3) as eg, \
             tc.tile_pool(name="qx", bufs=2, space="PSUM") as qx, \
             tc.tile_pool(name="ep", bufs=1, space="PSUM") as ep, \
             tc.tile_pool(name="yp", bufs=2, space="PSUM") as yp:
            for b in range(NBLK):
                T0 = b * TBLK
                kv_g = eg.tile([P, TBLK, 2 * D], f16, name="kv_g")
                for t in range(TBLK):
                    nc.gpsimd.indirect_dma_start(
                        out=kv_g[:, t, :], out_offset=None, in_=kv_d[:],
                        in_offset=bass.IndirectOffsetOnAxis(
                            ap=ci_sb[:, T0 + t:T0 + t + 1], axis=0))
                selt_b = eg.tile([P, TBLK * P], f16, name="selt_b")
                nc.sync.dma_start(selt_b[:], selt_d[b, :, :])
                qb = eg.tile([P, D], f16, name="qb")
                nc.sync.dma_start(qb[:], q_d[b * P:(b + 1) * P, :])

                sel = eg.tile([P, TBLK, P], bf16, name="sel")
                nc.vector.tensor_tensor(
                    out=sel[:],
                    in0=rl_sb[:, T0:T0 + TBLK].to_broadcast((P, TBLK, P)),
                    in1=io_sb[:][:, None, :].to_broadcast((P, TBLK, P)),
                    op=OP.is_equal)
                prod = eg.tile([P, TBLK, D], f16, name="prod")
                for t in range(TBLK):
                    qx_ps = qx.tile([P, D], f32, name="qx_ps")
                    nc.tensor.matmul(qx_ps[:],
                                     lhsT=selt_b[:, t * P:(t + 1) * P],
                                     rhs=qb[:], start=True, stop=True)
                    nc.vector.tensor_tensor(out=prod[:, t, :], in0=qx_ps[:],
                                            in1=kv_g[:, t, 0:D], op=OP.mult)
                s_b = eg.tile([P, TBLK, H], f32, name="s_b")
                nc.vector.tensor_reduce(
                    out=s_b[:],
                    in_=prod[:].rearrange("p t (h d) -> p t h d", h=H),
                    axis=mybir.AxisListType.X, op=OP.add)
                wext = eg.tile([P, TBLK, D + H], bf16, name="wext")
                nc.scalar.activation(wext[:, :, D:D + H], s_b[:], AF.Exp)
                nc.vector.tensor_tensor(
                    out=wext[:, :, 0:D].rearrange("p t (h d) -> p t h d", h=H),
                    in0=kv_g[:, :, D:2 * D].rearrange(
                        "p t (h d) -> p t h d", h=H),
                    in1=wext[:, :, D:D + H].to_broadcast((P, TBLK, H, DH)),
                    op=OP.mult)

                ypre = yp.tile([P, D + H], f32, name="ypre")
                for t in range(TBLK):
                    nc.tensor.matmul(ypre[:], lhsT=sel[:, t, :],
                                     rhs=wext[:, t, :],
                                     start=(t == 0), stop=(t == TBLK - 1))

                zr = eg.tile([P, H], f32, name="zr")
                nc.vector.tensor_scalar_add(zr[:], ypre[:, D:D + H], 1e-30)
                rz = eg.tile([P, H], f32, name="rz")
                nc.vector.reciprocal(rz[:], zr[:])
                yb = eg.tile([P, D], f16, name="yb")
                nc.vector.tensor_tensor(
                    out=yb[:].rearrange("p (h d) -> p h d", h=H),
                    in0=ypre[:, 0:D].rearrange("p (h d) -> p h d", h=H),
                    in1=rz[:].to_broadcast((P, H, DH)),
                    op=OP.mult)
                yT_ps = ep.tile([P, D], f16, name="yT_ps")
                nc.tensor.transpose(yT_ps[:], yb[:], ident[:])
                yT = eg.tile([P, D], f16, name="yT")
                nc.scalar.copy(yT[:], yT_ps[:])
                o_ps = ep.tile([P, D], f32, name="o_ps")
                nc.tensor.matmul(o_ps[:], lhsT=ones_sb[:], rhs=bo_sb[:],
                                 start=True, stop=False)
                nc.tensor.matmul(o_ps[:], lhsT=yT[:], rhs=wo_sb[:],
                                 start=False, stop=True)
                o_sb = eg.tile([P, D], f32, name="o_sb")
                nc.scalar.copy(o_sb[:], o_ps[:])
                nc.scalar.dma_start(out_d[b * P:(b + 1) * P, :], o_sb[:])

    nc.compile()
    return nc


def _prepare_inputs(x, row, col, Wq, bq, Wk, bk, Wv, bv, Wo, bo,
                    NPAD, NOWN, NBLK, TBLK):
    """Host-side sharding: per-core padded edge lists + permuted weights."""
    N = x.shape[0]
    perm = _channel_perm()
    s = np.sqrt(float(H))
    wkv_in = np.ascontiguousarray(
        np.concatenate([Wk[perm, :].T, Wv[perm, :].T], axis=1)
    ).astype(np.float16)
    wq_in = np.ascontiguousarray((Wq[perm, :] / s).T).astype(np.float16)
    wo_in = np.ascontiguousarray(Wo[:, perm].T).astype(np.float16)
    bq_in = (bq[perm] / s).reshape(1, D).astype(np.float16)
    # bv folds through the output projection exactly: sum_e a_e = 1.
    bo_in = (bo + Wo @ bv).reshape(1, D).astype(np.float16)
    io_in = np.tile(np.arange(P, dtype=np.float16), (P, 1))

    x_pad = np.zeros((NPAD, D), np.float32)
    x_pad[:N] = x
    xt_in = np.ascontiguousarray(x_pad.T)

    NT = NBLK * TBLK
    EPC = NT * P  # padded edges per core
    in_maps = []
    for c in range(NCORES):
        lo, hi = c * NOWN, (c + 1) * NOWN
        e0 = np.searchsorted(row, lo, "left")
        e1 = np.searchsorted(row, hi, "left")
        rows_c = (row[e0:e1] - lo).astype(np.int64)
        cols_c = col[e0:e1].astype(np.int64)
        blk = rows_c // P
        blk_starts = np.searchsorted(blk, np.arange(NBLK), "left")
        rank = np.arange(rows_c.shape[0]) - blk_starts[blk]
        cnts = np.bincount(blk, minlength=NBLK)
        if cnts.max() > TBLK * P:
            raise ValueError(f"TBLK={TBLK} too small: need "
                             f"{int(np.ceil(cnts.max() / P))}")
        pos = blk * (TBLK * P) + rank
        ci = np.zeros(EPC, np.int32)
        rl = np.full(EPC, -1.0, np.float16)
        ci[pos] = cols_c.astype(np.int32)
        rl[pos] = (rows_c % P).astype(np.float16)
        # one-hot transposed selection matrices, host-built: selT[b, j, e]
        selt = np.zeros((NBLK, P, TBLK * P), np.float16)
        selt[blk, rows_c % P, rank] = 1.0
        in_maps.append({
            "xt": xt_in,
            "xot": np.ascontiguousarray(x_pad[lo:hi].T),
            "wkv": wkv_in, "wq": wq_in, "wo": wo_in,
            "bq": bq_in, "bo": bo_in,
            "ci": np.ascontiguousarray(ci.reshape(NT, P).T),
            "rl": np.ascontiguousarray(rl.reshape(NT, P).T),
            "io": io_in, "selt": selt,
        })
    return in_maps


def _required_tblk(row, NOWN, NBLK):
    row = np.asarray(row, np.int64)
    need = 1
    for c in range(NCORES):
        lo, hi = c * NOWN, (c + 1) * NOWN
        e0 = np.searchsorted(row, lo, "left")
        e1 = np.searchsorted(row, hi, "left")
        blk = (row[e0:e1] - lo) // P
        cnts = np.bincount(blk, minlength=NBLK)
        need = max(need, int(np.ceil(cnts.max() / P)))
    return need


def _install_ntff_hook():
    """The agent image's antenv lacks axon_hooks; inject it so trace=True
    can drive NTFF profiling through libaxon_pjrt.so."""
    import importlib
    try:
        importlib.import_module("antenv.axon_hooks")
        return
    except ImportError:
        pass
    import types
    if "/root/.axon_site" not in sys.path:
        sys.path.insert(0, "/root/.axon_site")
    from trn_agent_boot.trn_boot import _ntff_profile_via_ctypes
    hook = _ntff_profile_via_ctypes("/opt/axon/libaxon_pjrt.so")
    mod = types.ModuleType("antenv.axon_hooks")
    state = {"hook": hook}
    mod.get_axon_ntff_profile_hook = lambda: state["hook"]
    mod.set_axon_ntff_profile_hook = lambda h: state.update(hook=h)
    import antenv
    antenv.axon_hooks = mod
    sys.modules["antenv.axon_hooks"] = mod


def run(x, row, col, Wq, bq, Wk, bk, Wv, bv, Wo, bo, NBLK=NBLK_FULL,
        trace=False, tmpdir=None):
    from concourse import bass_utils
    from concourse.bass_utils import run_bass_kernel_spmd
    if trace:
        _install_ntff_hook()
        bass_utils.upload_artifacts = lambda d: "local://" + d

    x = np.asarray(x, np.float32)
    row = np.asarray(row, np.int64)
    col = np.asarray(col, np.int64)
    N = x.shape[0]
    NOWN = NBLK * P
    NPAD = NCORES * NOWN
    assert NPAD >= N
    TBLK = _required_tblk(row, NOWN, NBLK)
    nc = _build_program(NPAD, NOWN, NBLK, TBLK)
    in_maps = _prepare_inputs(
        x, row, col,
        np.asarray(Wq, np.float32), np.asarray(bq, np.float32),
        np.asarray(Wk, np.float32), np.asarray(bk, np.float32),
        np.asarray(Wv, np.float32), np.asarray(bv, np.float32),
        np.asarray(Wo, np.float32), np.asarray(bo, np.float32),
        NPAD, NOWN, NBLK, TBLK)
    res = run_bass_kernel_spmd(nc, in_maps, list(range(NCORES)), trace=trace,
                               tmpdir=tmpdir)
    out = np.concatenate([res.results[c]["out"] for c in range(NCORES)], 0)
    return out[:N].astype(np.float32), res


def kernel(**inputs):
    out, _ = run(**inputs)
    return out


# revision 17
# speedup vs baseline: 2.1516x; 1.0003x over previous
import sys

sys.path.insert(0, "/opt/trn_rl_repo")

import numpy as np

P = 128          # partitions / tile edge
D = 128          # model dim
H = 4            # heads
DH = 32          # head dim
NCORES = 8

# Full-problem geometry (N=100000, E=800000). Each core owns NBLK node
# blocks of 128 nodes; every block's incident-edge list is padded to
# TBLK tiles of 128 edges so the SPMD program is uniform across cores.
NBLK_FULL = 98                      # 98*128 = 12544 own nodes/core
NPAD_FULL = NCORES * NBLK_FULL * P  # 100352 padded nodes


def _channel_perm():
    # torch reshape (N, DH, H): flat channel c = d*H + h. We relayout to
    # h-major c' = h*DH + d by permuting weight rows: perm[c'] = d*H + h.
    cp = np.arange(D)
    return (cp % DH) * H + (cp // DH)


def _build_program(NPAD, NOWN, NBLK, TBLK):
    import concourse.bass as bass
    import concourse.tile as tile
    from concourse import bacc, mybir
    from concourse.masks import make_identity
    from contextlib import ExitStack

    dt = mybir.dt
    f32, f16, bf16, i32 = dt.float32, dt.float16, dt.bfloat16, dt.int32
    NT = NBLK * TBLK      # edge tiles per core
    XT = NPAD // P        # x tiles for k/v projection (all nodes)
    QT = NOWN // P        # x tiles for q projection (own nodes) == NBLK

    nc = bacc.Bacc("TRN2", target_bir_lowering=False, debug=False,
                   num_devices=NCORES)

    # x ships host-transposed (channel-major) so the contraction dim is
    # already on partitions: no PE transpose needed anywhere.
    xt_d = nc.dram_tensor("xt", [D, NPAD], f16, kind="ExternalInput").ap()
    xot_d = nc.dram_tensor("xot", [D, NOWN], f16, kind="ExternalInput").ap()
    wkv_d = nc.dram_tensor("wkv", [D, 2 * D], f16, kind="ExternalInput").ap()
    wq_d = nc.dram_tensor("wq", [D, D], f16, kind="ExternalInput").ap()
    wo_d = nc.dram_tensor("wo", [D, D], f16, kind="ExternalInput").ap()
    bq_d = nc.dram_tensor("bq", [1, D], f16, kind="ExternalInput").ap()
    bo_d = nc.dram_tensor("bo", [1, D], f16, kind="ExternalInput").ap()
    ci_d = nc.dram_tensor("ci", [P, NT], i32, kind="ExternalInput").ap()
    selt_d = nc.dram_tensor("selt", [NBLK, P, TBLK * P], f16,
                            kind="ExternalInput").ap()
    rl_d = nc.dram_tensor("rl", [P, NT], f16, kind="ExternalInput").ap()
    io_d = nc.dram_tensor("io", [P, P], f16, kind="ExternalInput").ap()

    out_d = nc.dram_tensor("out", [NOWN, D], f32, kind="ExternalOutput").ap()
    kv_d = nc.dram_tensor("kv", [NPAD, 2 * D], f16).ap()
    q_d = nc.dram_tensor("q", [NOWN, D], f16).ap()


    AF = mybir.ActivationFunctionType
    OP = mybir.AluOpType

    with tile.TileContext(nc) as tc, ExitStack() as ctx:
        res = ctx.enter_context(tc.tile_pool(name="res", bufs=1))
        wkv_sb = res.tile([D, 2 * D], f16, name="wkv_sb")
        wq_sb = res.tile([D, D], f16, name="wq_sb")
        wo_sb = res.tile([D, D], f16, name="wo_sb")
        bq_sb = res.tile([1, D], f16, name="bq_sb")
        bo_sb = res.tile([1, D], f16, name="bo_sb")
        ci_sb = res.tile([P, NT], i32, name="ci_sb")

        rl_sb = res.tile([P, NT], f16, name="rl_sb")
        io_sb = res.tile([P, P], f16, name="io_sb")
        ones_sb = res.tile([1, P], f16, name="ones_sb")
        ident = res.tile([P, P], f16, name="ident")

        for sb_t, dr_t in [(wkv_sb, wkv_d), (wq_sb, wq_d), (wo_sb, wo_d),
                           (bq_sb, bq_d), (bo_sb, bo_d), (ci_sb, ci_d),
                           (rl_sb, rl_d), (io_sb, io_d)]:
            nc.sync.dma_start(sb_t[:], dr_t[:])
        nc.vector.memset(ones_sb[:], 1.0)
        make_identity(nc, ident[:])

        CH = 4  # x tiles per DMA chunk
        with tc.tile_pool(name="xa", bufs=3) as xa, \
             tc.tile_pool(name="pa", bufs=2, space="PSUM") as pa:
            # ---- q projection first: q_d is ready early so the q-gather
            # staging below overlaps the (long) kv projection loop.
            for j0 in range(0, QT, CH):
                c = min(CH, QT - j0)
                xo16 = xa.tile([P, c * P], f16, name="xo16")
                nc.sync.dma_start(xo16[:], xot_d[:, j0 * P:(j0 + c) * P])
                q4 = xa.tile([P, c, D], f16, name="q4")
                for t in range(c):
                    q_ps = pa.tile([P, D], f32, name="q_ps")
                    nc.tensor.matmul(q_ps[:], lhsT=ones_sb[:], rhs=bq_sb[:],
                                     start=True, stop=False)
                    nc.tensor.matmul(q_ps[:],
                                     lhsT=xo16[:, t * P:(t + 1) * P],
                                     rhs=wq_sb[:], start=False, stop=True)
                    nc.scalar.copy(q4[:, t, :], q_ps[:])
                nc.scalar.dma_start(
                    q_d[j0 * P:(j0 + c) * P, :].rearrange(
                        "(t p) c -> p t c", p=P), q4[:])

            # ---- kv projection for all nodes ----
            for i0 in range(0, XT, CH):
                c = min(CH, XT - i0)
                x16 = xa.tile([P, c * P], f16, name="x16")
                nc.sync.dma_start(x16[:], xt_d[:, i0 * P:(i0 + c) * P])
                kv4 = xa.tile([P, c, 2 * D], f16, name="kv4")
                for t in range(c):
                    kv_ps = pa.tile([P, 2 * D], f32, name="kv_ps")
                    nc.tensor.matmul(kv_ps[:],
                                     lhsT=x16[:, t * P:(t + 1) * P],
                                     rhs=wkv_sb[:], start=True, stop=True)
                    # split PSUM evacuation across the two free engines
                    if t % 2 == 0:
                        nc.vector.tensor_copy(kv4[:, t, :], kv_ps[:])
                    else:
                        nc.scalar.copy(kv4[:, t, :], kv_ps[:])
                nc.scalar.dma_start(
                    kv_d[i0 * P:(i0 + c) * P, :].rearrange(
                        "(t p) c -> p t c", p=P), kv4[:])

        # ---- phase B: per-block kv gather + scores + aggregation ----
        with tc.tile_pool(name="eg", bufs=3) as eg, \
             tc.tile_pool(name="qx", bufs=2, space="PSUM") as qx, \
             tc.tile_pool(name="ep", bufs=1, space="PSUM") as ep, \
             tc.tile_pool(name="yp", bufs=2, space="PSUM") as yp:
            for b in range(NBLK):
                T0 = b * TBLK
                kv_g = eg.tile([P, TBLK, 2 * D], f16, name="kv_g")
                for t in range(TBLK):
                    nc.gpsimd.indirect_dma_start(
                        out=kv_g[:, t, :], out_offset=None, in_=kv_d[:],
                        in_offset=bass.IndirectOffsetOnAxis(
                            ap=ci_sb[:, T0 + t:T0 + t + 1], axis=0))
                selt_b = eg.tile([P, TBLK * P], f16, name="selt_b")
                nc.sync.dma_start(selt_b[:], selt_d[b, :, :])
                qb = eg.tile([P, D], f16, name="qb")
                nc.sync.dma_start(qb[:], q_d[b * P:(b + 1) * P, :])

                sel = eg.tile([P, TBLK, P], bf16, name="sel")
                nc.vector.tensor_tensor(
                    out=sel[:],
                    in0=rl_sb[:, T0:T0 + TBLK].to_broadcast((P, TBLK, P)),
                    in1=io_sb[:][:, None, :].to_broadcast((P, TBLK, P)),
                    op=OP.is_equal)
                prod = eg.tile([P, TBLK, D], f16, name="prod")
                for t in range(TBLK):
                    qx_ps = qx.tile([P, D], f32, name="qx_ps")
                    nc.tensor.matmul(qx_ps[:],
                                     lhsT=selt_b[:, t * P:(t + 1) * P],
                                     rhs=qb[:], start=True, stop=True)
                    nc.vector.tensor_tensor(out=prod[:, t, :], in0=qx_ps[:],
                                            in1=kv_g[:, t, 0:D], op=OP.mult)
                s_b = eg.tile([P, TBLK, H], f32, name="s_b")
                nc.vector.tensor_reduce(
                    out=s_b[:],
                    in_=prod[:].rearrange("p t (h d) -> p t h d", h=H),
                    axis=mybir.AxisListType.X, op=OP.add)
                wext = eg.tile([P, TBLK, D + H], bf16, name="wext")
                nc.scalar.activation(wext[:, :, D:D + H], s_b[:], AF.Exp)
                nc.vector.tensor_tensor(
                    out=wext[:, :, 0:D].rearrange("p t (h d) -> p t h d", h=H),
                    in0=kv_g[:, :, D:2 * D].rearrange(
                        "p t (h d) -> p t h d", h=H),
                    in1=wext[:, :, D:D + H].to_broadcast((P, TBLK, H, DH)),
                    op=OP.mult)

                ypre = yp.tile([P, D + H], f32, name="ypre")
                for t in range(TBLK):
                    nc.tensor.matmul(ypre[:], lhsT=sel[:, t, :],
                                     rhs=wext[:, t, :],
                                     start=(t == 0), stop=(t == TBLK - 1))

                zr = eg.tile([P, H], f32, name="zr")
                nc.vector.tensor_scalar_add(zr[:], ypre[:, D:D + H], 1e-30)
                rz = eg.tile([P, H], f32, name="rz")
                nc.vector.reciprocal(rz[:], zr[:])
                yb = eg.tile([P, D], f16, name="yb")
                nc.vector.tensor_tensor(
                    out=yb[:].rearrange("p (h d) -> p h d", h=H),
                    in0=ypre[:, 0:D].rearrange("p (h d) -> p h d", h=H),
                    in1=rz[:].to_broadcast((P, H, DH)),
                    op=OP.mult)
                yT_ps = ep.tile([P, D], f16, name="yT_ps")
                nc.tensor.transpose(yT_ps[:], yb[:], ident[:])
                yT = eg.tile([P, D], f16, name="yT")
                nc.scalar.copy(yT[:], yT_ps[:])
                o_ps = ep.tile([P, D], f32, name="o_ps")
                nc.tensor.matmul(o_ps[:], lhsT=ones_sb[:], rhs=bo_sb[:],
                                 start=True, stop=False)
                nc.tensor.matmul(o_ps[:], lhsT=yT[:], rhs=wo_sb[:],
                                 start=False, stop=True)
                o_sb = eg.tile([P, D], f32, name="o_sb")
                nc.scalar.copy(o_sb[:], o_ps[:])
                nc.scalar.dma_start(out_d[b * P:(b + 1) * P, :], o_sb[:])

    nc.compile()
    return nc


def _prepare_inputs(x, row, col, Wq, bq, Wk, bk, Wv, bv, Wo, bo,
                    NPAD, NOWN, NBLK, TBLK):
    """Host-side sharding: per-core padded edge lists + permuted weights."""
    N = x.shape[0]
    perm = _channel_perm()
    s = np.sqrt(float(H))
    wkv_in = np.ascontiguousarray(
        np.concatenate([Wk[perm, :].T, Wv[perm, :].T], axis=1)
    ).astype(np.float16)
    wq_in = np.ascontiguousarray((Wq[perm, :] / s).T).astype(np.float16)
    wo_in = np.ascontiguousarray(Wo[:, perm].T).astype(np.float16)
    bq_in = (bq[perm] / s).reshape(1, D).astype(np.float16)
    # bv folds through the output projection exactly: sum_e a_e = 1.
    bo_in = (bo + Wo @ bv).reshape(1, D).astype(np.float16)
    io_in = np.tile(np.arange(P, dtype=np.float16), (P, 1))

    x_pad = np.zeros((NPAD, D), np.float32)
    x_pad[:N] = x
    xt_in = np.ascontiguousarray(x_pad.T).astype(np.float16)

    NT = NBLK * TBLK
    EPC = NT * P  # padded edges per core
    in_maps = []
    for c in range(NCORES):
        lo, hi = c * NOWN, (c + 1) * NOWN
        e0 = np.searchsorted(row, lo, "left")
        e1 = np.searchsorted(row, hi, "left")
        rows_c = (row[e0:e1] - lo).astype(np.int64)
        cols_c = col[e0:e1].astype(np.int64)
        blk = rows_c // P
        blk_starts = np.searchsorted(blk, np.arange(NBLK), "left")
        rank = np.arange(rows_c.shape[0]) - blk_starts[blk]
        cnts = np.bincount(blk, minlength=NBLK)
        if cnts.max() > TBLK * P:
            raise ValueError(f"TBLK={TBLK} too small: need "
                             f"{int(np.ceil(cnts.max() / P))}")
        pos = blk * (TBLK * P) + rank
        ci = np.zeros(EPC, np.int32)
        rl = np.full(EPC, -1.0, np.float16)
        ci[pos] = cols_c.astype(np.int32)
        rl[pos] = (rows_c % P).astype(np.float16)
        # one-hot transposed selection matrices, host-built: selT[b, j, e]
        selt = np.zeros((NBLK, P, TBLK * P), np.float16)
        selt[blk, rows_c % P, rank] = 1.0
        in_maps.append({
            "xt": xt_in,
            "xot": np.ascontiguousarray(x_pad[lo:hi].T).astype(np.float16),
            "wkv": wkv_in, "wq": wq_in, "wo": wo_in,
            "bq": bq_in, "bo": bo_in,
            "ci": np.ascontiguousarray(ci.reshape(NT, P).T),
            "rl": np.ascontiguousarray(rl.reshape(NT, P).T),
            "io": io_in, "selt": selt,
        })
    return in_maps


def _required_tblk(row, NOWN, NBLK):
    row = np.asarray(row, np.int64)
    need = 1
    for c in range(NCORES):
        lo, hi = c * NOWN, (c + 1) * NOWN
        e0 = np.searchsorted(row, lo, "left")
        e1 = np.searchsorted(row, hi, "left")
        blk = (row[e0:e1] - lo) // P
        cnts = np.bincount(blk, minlength=NBLK)
        need = max(need, int(np.ceil(cnts.max() / P)))
    return need


def _install_ntff_hook():
    """The agent image's antenv lacks axon_hooks; inject it so trace=True
    can drive NTFF profiling through libaxon_pjrt.so."""
    import importlib
    try:
        importlib.import_module("antenv.axon_hooks")
        return
    except ImportError:
        pass
    import types
    if "/root/.axon_site" not in sys.path:
        sys.path.insert(0, "/root/.axon_site")
    from trn_agent_boot.trn_boot import _ntff_profile_via_ctypes
    hook = _ntff_profile_via_ctypes("/opt/axon/libaxon_pjrt.so")
    mod = types.ModuleType("antenv.axon_hooks")
    state = {"hook": hook}
    mod.get_axon_ntff_profile_hook = lambda: state["hook"]
    mod.set_axon_ntff_profile_hook = lambda h: state.update(hook=h)
    import antenv
    antenv.axon_hooks = mod
    sys.modules["antenv.axon_hooks"] = mod


def run(x, row, col, Wq, bq, Wk, bk, Wv, bv, Wo, bo, NBLK=NBLK_FULL,
        trace=False, tmpdir=None):
    from concourse import bass_utils
    from concourse.bass_utils import run_bass_kernel_spmd
    if trace:
        _install_ntff_hook()
        bass_utils.upload_artifacts = lambda d: "local://" + d

    x = np.asarray(x, np.float32)
    row = np.asarray(row, np.int64)
    col = np.asarray(col, np.int64)
    N = x.shape[0]
    NOWN = NBLK * P
    NPAD = NCORES * NOWN
    assert NPAD >= N
    TBLK = _required_tblk(row, NOWN, NBLK)
    nc = _build_program(NPAD, NOWN, NBLK, TBLK)
    in_maps = _prepare_inputs(
        x, row, col,
        np.asarray(Wq, np.float32), np.asarray(bq, np.float32),
        np.asarray(Wk, np.float32), np.asarray(bk, np.float32),
        np.asarray(Wv, np.float32), np.asarray(bv, np.float32),
        np.asarray(Wo, np.float32), np.asarray(bo, np.float32),
        NPAD, NOWN, NBLK, TBLK)
    res = run_bass_kernel_spmd(nc, in_maps, list(range(NCORES)), trace=trace,
                               tmpdir=tmpdir)
    out = np.concatenate([res.results[c]["out"] for c in range(NCORES)], 0)
    return out[:N].astype(np.float32), res


def kernel(**inputs):
    out, _ = run(**inputs)
    return out
